# revision 1
# baseline (speedup 1.0000x reference)
"""Trainium2 Bass kernel for DynamicTokenMixing (16-head attention, N=4096, C=1024).

Sharding: head-parallel across 8 NeuronCores, 2 heads per core. Each core
computes q/k/v projections for its 2 heads, full attention for those heads,
and a partial output projection (row-parallel over Wproj); the host sums the
8 partials and adds the bias.

Per-core dataflow (all layouts chosen so no on-chip transposes of the
attention matrix are needed):
  qT, kT   = (x @ Wq_cols).T, (x @ Wkv_kcols).T     [dual-head stacked, 128 x 4096]
  vT       = (x @ Wkv_vcols).T  -> PE-transposed to v tiles [j, d] (+ ones col)
  ST[j,i]  = sum_d k[j,d] q[i,d]          (scores, transposed layout)
  ET       = exp(0.5 * ST)                (0.5 = gpd_ratio^2 * d^-0.5)
  AV^T     = sum_j v_ext[j,:]^T ET[j,:]   (row 64 = softmax denominator l[i])
  outT     = AV^T[0:64] * (1/l) (broadcast)
  out      = sum_h outT_h.T @ Wproj_rows_h   (partial; host adds across cores)
"""

import numpy as np

import concourse.bass as bass
import concourse.mybir as mybir
import concourse.tile as tile
from concourse import bacc
from concourse.bass_utils import run_bass_kernel_spmd
from concourse.masks import make_identity

F32 = mybir.dt.float32
F32R = mybir.dt.float32r
F16 = mybir.dt.float16
BF16 = mybir.dt.bfloat16

N = 4096          # tokens
C = 1024          # model dim
D = 64            # head dim
NHEADS = 16
GPD = 2
NCORES = 8
NJ = N // 128     # 32 key tiles
NCT = C // 128    # 8 contraction tiles
STRIP = 512       # query-strip width
NSTRIP = N // STRIP
JB = 2            # key tiles batched per exp instruction
SCORE_SCALE = GPD * GPD * (D ** -0.5)  # 0.5


def build_nc(repeat=1, hw_loop=False):
    nc = bacc.Bacc("TRN2", target_bir_lowering=False, debug=False,
                   num_devices=NCORES)
    xT = nc.declare_dram_parameter("xT", [C, N], F32R, isOutput=False)
    wq = nc.declare_dram_parameter("wq", [C, 128], F32R, isOutput=False)
    wk = nc.declare_dram_parameter("wk", [C, 128], F32R, isOutput=False)
    wv = nc.declare_dram_parameter("wv", [C, 128], F32R, isOutput=False)
    wpa = nc.declare_dram_parameter("wpa", [D, C], F32R, isOutput=False)
    wpb = nc.declare_dram_parameter("wpb", [D, C], F32R, isOutput=False)
    out = nc.declare_dram_parameter("out", [N, C], F32, isOutput=True)

    xT_r = xT[:].rearrange("(t p) n -> p t n", p=128)    # [128, 8, 4096]
    out_r = out[:].rearrange("(t p) o -> t p o", p=128)  # [32, 128, 1024]

    with tile.TileContext(nc) as tc:
        with (
            nc.allow_low_precision(reason="fp32r (tf32) matmul inputs by design"),
            tc.tile_pool(name="persist", bufs=1) as persist,
            tc.tile_pool(name="small", bufs=4) as small,
        ):
            wq_sb = persist.tile([128, NCT, 128], F32R)
            wk_sb = persist.tile([128, NCT, 128], F32R)
            wv_sb = persist.tile([128, NCT, 128], F32R)
            wpa_sb = persist.tile([D, C], F32R)
            wpb_sb = persist.tile([D, C], F32R)
            # per-strip tiles so dependencies are fine-grained (phase overlap)
            qT_s = [persist.tile([128, STRIP], F32R, name=f"qT{i}")
                    for i in range(NSTRIP)]
            kT_s = [persist.tile([128, STRIP], F32R, name=f"kT{i}")
                    for i in range(NSTRIP)]
            vT_s = [persist.tile([128, STRIP], F32, name=f"vT{i}")
                    for i in range(NSTRIP)]
            # per-key-tile v in natural layout: [j, (vA|1|vB|1)]
            vsb = [persist.tile([128, 130], F32R, name=f"vsb{j}")
                   for j in range(NJ)]
            outT = {h: [persist.tile([D, STRIP], F32R, name=f"outT{h}_{i}")
                        for i in range(NSTRIP)] for h in (0, 1)}
            ident = persist.tile([128, 128], F32)
            ones_f = persist.tile([128, D], F32)
            nc.gpsimd.memset(ones_f[:], 1.0)
            ones_t = persist.tile([65, D], F32R)
            nc.vector.tensor_copy(ones_t[:], ones_f[0:65, :])

            nc.sync.dma_start(wq_sb[:], wq[:].rearrange("(t p) m -> p t m", p=128))
            nc.sync.dma_start(wk_sb[:], wk[:].rearrange("(t p) m -> p t m", p=128))
            nc.sync.dma_start(wv_sb[:], wv[:].rearrange("(t p) m -> p t m", p=128))
            nc.sync.dma_start(wpa_sb[:], wpa[:])
            nc.sync.dma_start(wpb_sb[:], wpb[:])
            make_identity(nc, ident[:])
            for j in range(NJ):
                nc.vector.tensor_copy(vsb[j][:, 64:65], ones_f[:, 0:1])
                nc.vector.tensor_copy(vsb[j][:, 129:130], ones_f[:, 0:1])

            import contextlib
            rep_iter = ([None] if hw_loop and repeat > 1 else range(repeat))
            for _rep in rep_iter:
              with (tc.For_i(0, repeat, 1) if hw_loop and repeat > 1
                    else contextlib.nullcontext()):
                  with (
                      tc.tile_pool(name="ph1_sb", bufs=2) as ph1_sb,
                      tc.tile_pool(name="ph1_ps", bufs=2, space="PSUM") as ph1_ps,
                      tc.tile_pool(name="tp_ps", bufs=2, space="PSUM") as tp_ps,
                  ):
                      # ---- Phase 1: qT/kT/vT projections; vsb natural-layout tiles ----
                      for i in range(NSTRIP):
                          sl = bass.ts(i, STRIP)
                          xt = ph1_sb.tile([128, NCT, STRIP], F32R, tag="xt")
                          nc.sync.dma_start(xt[:], xT_r[:, :, sl])
                          q_ps = ph1_ps.tile([128, STRIP], F32, tag="q")
                          k_ps = ph1_ps.tile([128, STRIP], F32, tag="k")
                          v_ps = ph1_ps.tile([128, STRIP], F32, tag="v")
                          for c in range(NCT):
                              st, sp = (c == 0), (c == NCT - 1)
                              nc.tensor.matmul(q_ps[:], wq_sb[:, c, :], xt[:, c, :],
                                               start=st, stop=sp)
                              nc.tensor.matmul(k_ps[:], wk_sb[:, c, :], xt[:, c, :],
                                               start=st, stop=sp)
                              nc.tensor.matmul(v_ps[:], wv_sb[:, c, :], xt[:, c, :],
                                               start=st, stop=sp)
                          nc.vector.tensor_copy(qT_s[i][:], q_ps[:])
                          nc.vector.tensor_copy(kT_s[i][:], k_ps[:])
                          nc.vector.tensor_copy(vT_s[i][:], v_ps[:])
                          for jj in range(STRIP // 128):
                              j = i * (STRIP // 128) + jj
                              tp = tp_ps.tile([128, 128], F32, tag="tp")
                              nc.tensor.transpose(tp[:], vT_s[i][:, bass.ts(jj, 128)],
                                                  ident[:])
                              nc.vector.tensor_copy(vsb[j][:, 0:64], tp[:, 0:64])
                              nc.vector.tensor_copy(vsb[j][:, 65:129], tp[:, 64:128])

                  # ---- Phase 2+3: attention + projection, pipelined per strip ----
                  with (
                      tc.tile_pool(name="att_et", bufs=3) as et_pool,
                      tc.tile_pool(name="pr_sb", bufs=2) as pr_sb,
                      tc.tile_pool(name="att_st", bufs=1, space="PSUM") as st_pool,
                      tc.tile_pool(name="att_av", bufs=1, space="PSUM") as av_pool,
                      tc.tile_pool(name="att_bc", bufs=1, space="PSUM") as bc_pool,
                      tc.tile_pool(name="pr_ps", bufs=1, space="PSUM") as pr_ps,
                  ):
                      heads = ((0, slice(0, 64)), (1, slice(64, 128)))
                      for i in range(NSTRIP):
                          av = {h: av_pool.tile([65, STRIP], F32, tag=f"av{h}",
                                                name=f"av{h}")
                                for h, _ in heads}
                          for jp in range(NJ // JB):
                              for h, hs in heads:
                                  st = st_pool.tile([128, JB * STRIP], F32, tag=f"st{h}")
                                  for u in range(JB):
                                      j = JB * jp + u
                                      nc.tensor.matmul(
                                          st[:, bass.ts(u, STRIP)],
                                          kT_s[j // (STRIP // 128)][hs, bass.ts(
                                              j % (STRIP // 128), 128)],
                                          qT_s[i][hs, :],
                                          start=True, stop=True,
                                      )
                                  et = et_pool.tile([128, JB * STRIP], F32R, tag=f"et{h}")
                                  nc.scalar.activation(
                                      et[:], st[:],
                                      mybir.ActivationFunctionType.Exp,
                                      scale=SCORE_SCALE,
                                  )
                                  for u in range(JB):
                                      j = JB * jp + u
                                      nc.tensor.matmul(
                                          av[h][:],
                                          vsb[j][:, h * 65:h * 65 + 65],
                                          et[:, bass.ts(u, STRIP)],
                                          start=(j == 0), stop=(j == NJ - 1),
                                          skip_group_check=True,
                                      )
                          for h, _ in heads:
                              stage = small.tile([65, STRIP], F32, tag="stage")
                              nc.vector.tensor_copy(stage[:], av[h][:])
                              rec_r = small.tile([65, STRIP], F32R, tag="rec_r")
                              nc.vector.reciprocal(rec_r[64:65, :], stage[64:65, :])
                              bc = bc_pool.tile([64, STRIP], F32, tag="bc")
                              nc.tensor.matmul(bc[:], ones_t[64:65, :],
                                               rec_r[64:65, :], start=True, stop=True)
                              nc.vector.tensor_mul(outT[h][i][:], stage[0:64, :], bc[:])
                          # projection for this strip's 4 row-tiles
                          for t in range(STRIP // 128):
                              it = i * (STRIP // 128) + t
                              tsl = bass.ts(t, 128)
                              ob = pr_sb.tile([128, C], F32, tag="ob")
                              for oc in range(C // STRIP):
                                  osl = bass.ts(oc, STRIP)
                                  pp = pr_ps.tile([128, STRIP], F32, tag="pp")
                                  nc.tensor.matmul(pp[:], outT[0][i][:, tsl],
                                                   wpa_sb[:, osl], start=True, stop=False)
                                  nc.tensor.matmul(pp[:], outT[1][i][:, tsl],
                                                   wpb_sb[:, osl], start=False, stop=True)
                                  nc.vector.tensor_copy(ob[:, osl], pp[:])
                              nc.sync.dma_start(out_r[it], ob[:])
    nc.finalize()
    return nc


def _colk(h):
    base = h * D if h < 8 else 2 * 512 + (h - 8) * D
    return slice(base, base + D)


def _colv(h):
    base = 512 + h * D if h < 8 else 3 * 512 + (h - 8) * D
    return slice(base, base + D)


def make_in_maps(x, Wq, Wkv, Wproj):
    x = np.asarray(x, np.float32).reshape(N, C)
    Wq = np.asarray(Wq, np.float32)
    Wkv = np.asarray(Wkv, np.float32)
    Wproj = np.asarray(Wproj, np.float32)
    xT = np.ascontiguousarray(x.T)
    in_maps = []
    for core in range(NCORES):
        h0, h1 = 2 * core, 2 * core + 1
        in_maps.append({
            "xT": xT,
            "wq": np.ascontiguousarray(
                np.concatenate([Wq[:, h0 * D:(h0 + 1) * D],
                                Wq[:, h1 * D:(h1 + 1) * D]], axis=1)),
            "wk": np.ascontiguousarray(
                np.concatenate([Wkv[:, _colk(h0)], Wkv[:, _colk(h1)]], axis=1)),
            "wv": np.ascontiguousarray(
                np.concatenate([Wkv[:, _colv(h0)], Wkv[:, _colv(h1)]], axis=1)),
            "wpa": np.ascontiguousarray(Wproj[h0 * D:(h0 + 1) * D, :]),
            "wpb": np.ascontiguousarray(Wproj[h1 * D:(h1 + 1) * D, :]),
        })
    return in_maps


_NC = None


def _get_nc():
    global _NC
    if _NC is None:
        _NC = build_nc()
    return _NC


def run_spmd(in_maps, **kwargs):
    return run_bass_kernel_spmd(_get_nc(), in_maps, list(range(NCORES)), **kwargs)


def kernel(x, Wq, Wkv, Wproj, bproj, H=None, W=None, **_unused):
    in_maps = make_in_maps(x, Wq, Wkv, Wproj)
    res = run_spmd(in_maps)
    acc = np.zeros((N, C), np.float64)
    for r in res.results:
        acc += r["out"]
    out = acc.astype(np.float32) + np.asarray(bproj, np.float32)[None, :]
    return out.reshape(1, N, C)


if __name__ == "__main__":
    nc = build_nc()
    print("built ok")



# revision 9
# speedup vs baseline: 1.1443x; 1.1443x over previous
"""Trainium2 Bass kernel for DynamicTokenMixing (16-head attention, N=4096, C=1024).

Sharding: head-parallel across 8 NeuronCores, 2 heads per core. Each core
computes q/k/v projections for its 2 heads, full attention for those heads,
and a partial output projection; the host sums the 8 partials and adds bias.

v2: all big matmuls use fp16 inputs (1 PE cycle/row vs ~3 for fp32r's
fp32_mode=HIGH lowering), phase 2 is a single software-pipelined stream over
(pair-of-strips, head, key-tile) steps with 1024-wide exp instructions, and
softmax normalization is applied after the projection using per-partition
(per-token) reciprocals obtained via tiny K=1 matmul "transposes" of the
denominator row.

Per-core dataflow:
  qT, kT    = (x @ Wq_cols).T etc  [128 x 4096 fp16, dual-head stacked]
  vsb[j]    = v tiles in [token, (vA|1|vB|1)] layout (PE-transposed)
  st[j,i]   = sum_d k[j,d] q[i,d]        (scores, [key, query] layout, PSUM)
  et        = exp(0.5*st - 10)           (fp16; -10 bias cancels in av/l)
  av        = sum_j vsb[j]^T et[j,:]     (row 64 = denominator l)
  lT[t]     = K=1 matmul transposing l row into per-partition layout
  out_tile  = (stage_h0^T Wp_h0) * (1/l0) + (stage_h1^T Wp_h1) * (1/l1)
"""

import numpy as np
import ml_dtypes

import concourse.bass as bass
import concourse.mybir as mybir
import concourse.tile as tile
from concourse import bacc
from concourse.bass_utils import run_bass_kernel_spmd
from concourse.masks import make_identity

F32 = mybir.dt.float32
F16 = mybir.dt.float16
BF16 = mybir.dt.bfloat16

N = 4096          # tokens
C = 1024          # model dim
D = 64            # head dim
NHEADS = 16
GPD = 2
NCORES = 8
NCT = C // 128    # 8 contraction tiles
STRIP = 512       # phase-1 strip width
NSTRIP = N // STRIP          # 8
NPAIR = NSTRIP // 2          # 4 pairs of strips (1024 tokens each)
NJ = N // 128     # 32 key tiles
NBLK = NPAIR * 2  # 8 (pair, head) blocks
LAG = 5           # av lags st by LAG steps in the phase-2 pipeline
SCORE_SCALE = GPD * GPD * (D ** -0.5)  # 0.5


def build_nc(repeat=1, hw_loop=False):
    nc = bacc.Bacc("TRN2", target_bir_lowering=False, debug=False,
                   num_devices=NCORES)
    xt_d = nc.declare_dram_parameter("xt", [128, NCT, N], F16, isOutput=False)
    wq_d = nc.declare_dram_parameter("wq", [128, NCT, 128], F16, isOutput=False)
    wk_d = nc.declare_dram_parameter("wk", [128, NCT, 128], F16, isOutput=False)
    wv_d = nc.declare_dram_parameter("wv", [128, NCT, 128], F16, isOutput=False)
    wpa_d = nc.declare_dram_parameter("wpa", [D, C], BF16, isOutput=False)
    wpb_d = nc.declare_dram_parameter("wpb", [D, C], BF16, isOutput=False)
    out_d = nc.declare_dram_parameter("out", [N, C], F32, isOutput=True)

    out_r = out_d[:].rearrange("(t p) o -> t p o", p=128)  # [32, 128, 1024]
    Exp = mybir.ActivationFunctionType.Exp
    mult = mybir.AluOpType.mult
    add = mybir.AluOpType.add

    with tile.TileContext(nc) as tc:
        with (
            nc.allow_low_precision(reason="fp16 matmul inputs by design"),
            tc.tile_pool(name="persist", bufs=1) as persist,
        ):
            wq_sb = persist.tile([128, NCT, 128], F16)
            wk_sb = persist.tile([128, NCT, 128], F16)
            wv_sb = persist.tile([128, NCT, 128], F16)
            wpa_sb = persist.tile([D, C], BF16)
            wpb_sb = persist.tile([D, C], BF16)
            # per-pair q/k in [dual-head-d, strip-of-pair, 512] layout
            qT = [persist.tile([128, 2, STRIP], F16, name=f"qT{p}")
                  for p in range(NPAIR)]
            kT = [persist.tile([128, 2, STRIP], F16, name=f"kT{p}")
                  for p in range(NPAIR)]
            # per-key-tile v in natural layout: [j, (vA|1|vB|1)]
            vsb = [persist.tile([128, 130], BF16, name=f"vsb{j}")
                   for j in range(NJ)]
            ident = persist.tile([128, 128], BF16)
            one_sb = persist.tile([1, 1], F32)

            nc.sync.dma_start(wq_sb[:], wq_d[:])
            nc.sync.dma_start(wk_sb[:], wk_d[:])
            nc.sync.dma_start(wv_sb[:], wv_d[:])
            nc.sync.dma_start(wpa_sb[:], wpa_d[:])
            nc.sync.dma_start(wpb_sb[:], wpb_d[:])
            make_identity(nc, ident[:])
            nc.gpsimd.memset(one_sb[:], 1.0)
            for j in range(NJ):
                nc.gpsimd.memset(vsb[j][:, 64:65], 1.0)
                nc.gpsimd.memset(vsb[j][:, 129:130], 1.0)

            import contextlib
            rep_iter = ([None] if hw_loop and repeat > 1 else range(repeat))
            for _rep in rep_iter:
              with (tc.For_i(0, repeat, 1) if hw_loop and repeat > 1
                    else contextlib.nullcontext()):
                # ---- Phase 1: q/k/v projections + v transposes ----
                with (
                    tc.tile_pool(name="xt_sb", bufs=2) as xt_pool,
                    tc.tile_pool(name="vt_sb", bufs=2) as vt_pool,
                    tc.tile_pool(name="qkv_ps", bufs=2, space="PSUM") as qkv_ps,
                    tc.tile_pool(name="tp_ps", bufs=2, space="PSUM") as tp_ps,
                ):
                    for i in range(NSTRIP):
                        xt_sb = xt_pool.tile([128, NCT, STRIP], F16, tag="xt")
                        nc.sync.dma_start(xt_sb[:],
                                          xt_d[:, :, bass.ts(i, STRIP)])
                        q_ps = qkv_ps.tile([128, STRIP], F32, tag="q")
                        k_ps = qkv_ps.tile([128, STRIP], F32, tag="k")
                        v_ps = qkv_ps.tile([128, STRIP], F32, tag="v")
                        for c in range(NCT):
                            st_, sp_ = (c == 0), (c == NCT - 1)
                            nc.tensor.matmul(q_ps[:], wq_sb[:, c, :],
                                             xt_sb[:, c, :], start=st_, stop=sp_)
                            nc.tensor.matmul(k_ps[:], wk_sb[:, c, :],
                                             xt_sb[:, c, :], start=st_, stop=sp_)
                            nc.tensor.matmul(v_ps[:], wv_sb[:, c, :],
                                             xt_sb[:, c, :], start=st_, stop=sp_)
                        p2, u = i // 2, i % 2
                        nc.vector.tensor_copy(qT[p2][:, u, :], q_ps[:])
                        nc.vector.tensor_copy(kT[p2][:, u, :], k_ps[:])
                        vt_sb = vt_pool.tile([128, STRIP], BF16, tag="vt")
                        nc.vector.tensor_copy(vt_sb[:], v_ps[:])
                        for jj in range(4):
                            j = 4 * i + jj
                            tp = tp_ps.tile([128, 128], BF16, tag="tp")
                            nc.tensor.transpose(tp[:],
                                                vt_sb[:, bass.ts(jj, 128)],
                                                ident[:])
                            nc.vector.tensor_copy(vsb[j][:, 0:64], tp[:, 0:64])
                            nc.vector.tensor_copy(vsb[j][:, 65:129],
                                                  tp[:, 64:128])

                # ---- Phase 2: one pipelined stream over (pair, head, j) ----
                with (
                    tc.tile_pool(name="st_ps", bufs=2, space="PSUM") as st_pool,
                    tc.tile_pool(name="av_ps", bufs=1, space="PSUM") as av_pool,
                    tc.tile_pool(name="pp_ps", bufs=2, space="PSUM") as pp_pool,
                    tc.tile_pool(name="et_sb", bufs=LAG + 1) as et_pool,
                    tc.tile_pool(name="so_sb", bufs=2) as so_pool,
                    tc.tile_pool(name="sl_sb", bufs=2) as sl_pool,
                    tc.tile_pool(name="rec_sb", bufs=2) as rec_pool,
                    tc.tile_pool(name="ob_sb", bufs=3) as ob_pool,
                ):
                    NSTEP = NBLK * NJ  # 256
                    ets = {}           # step -> et tile
                    avs = {}           # block -> av tile (full 128 partitions)
                    stage_os = {}      # block -> stage_o tile
                    stage_ls = {}      # block -> stage_l tile
                    recs = {}          # block -> reciprocal tile [128, 8]
                    obs = {}           # (pair, t) -> output tile
                    spread = []        # deferred closures, popped 1/step

                    def emit_st(n):
                        b, j = n // NJ, n % NJ
                        pair, h = b // 2, b % 2
                        hs = slice(64 * h, 64 * h + 64)
                        stt = st_pool.tile([128, 2, STRIP], F32, tag="st")
                        kt = kT[j // 8][hs, (j // 4) % 2, bass.ts(j % 4, 128)]
                        for u2 in range(2):
                            nc.tensor.matmul(stt[:, u2, :], kt,
                                             qT[pair][hs, u2, :],
                                             start=True, stop=True)
                        et = et_pool.tile([128, 2, STRIP], BF16, tag="et")
                        nc.scalar.activation(et[:], stt[:], Exp,
                                             scale=SCORE_SCALE)
                        ets[n] = et

                    def emit_av(n):
                        b, j = n // NJ, n % NJ
                        h = b % 2
                        if j == 0:
                            avs[b] = av_pool.tile([128, 2, STRIP], F32,
                                                  tag="av", name=f"av{b}")
                        av = avs[b]
                        et = ets.pop(n)
                        lhs = vsb[j][:, 65 * h:65 * h + 65]
                        for u2 in range(2):
                            nc.tensor.matmul(av[0:65, u2, :], lhs,
                                             et[:, u2, :],
                                             start=(j == 0), stop=(j == NJ - 1),
                                             skip_group_check=True)

                    def emit_block_tail(b):
                        # runs right after av(b, NJ-1) is emitted; must finish
                        # before the next block's first av allocation so the
                        # av-pool rotation (av, lT, av, lT, ...) stays ordered
                        av = avs.pop(b)
                        sl = sl_pool.tile([1, 2, STRIP], F32, tag="sl")
                        nc.vector.tensor_copy(sl[:], av[64:65, :, :])
                        so = so_pool.tile([D, 2, STRIP], BF16, tag="so")
                        nc.vector.tensor_copy(so[:], av[0:64, :, :])
                        stage_os[b] = so
                        stage_ls[b] = sl
                        ltfull = av_pool.tile([128, 2, STRIP], F32, tag="av")
                        lt = ltfull[:, 0, 0:8]
                        for t in range(8):
                            nc.tensor.matmul(lt[:, t:t + 1],
                                             sl[0:1, t // 4, bass.ts(t % 4, 128)],
                                             one_sb[0:1, 0:1],
                                             start=True, stop=True)
                        rec = rec_pool.tile([128, 8], F32, tag="rec")
                        nc.vector.reciprocal(rec[:], lt[:])
                        recs[b] = rec

                    def make_proj(pair, t, oc):
                        def closure():
                            so0, so1 = stage_os[2 * pair], stage_os[2 * pair + 1]
                            rec0, rec1 = recs[2 * pair], recs[2 * pair + 1]
                            osl = bass.ts(oc, STRIP)
                            pp0 = pp_pool.tile([128, STRIP], F32, tag="pp")
                            nc.tensor.matmul(pp0[:],
                                             so0[:, t // 4, bass.ts(t % 4, 128)],
                                             wpa_sb[:, osl],
                                             start=True, stop=True)
                            pp1 = pp_pool.tile([128, STRIP], F32, tag="pp")
                            nc.tensor.matmul(pp1[:],
                                             so1[:, t // 4, bass.ts(t % 4, 128)],
                                             wpb_sb[:, osl],
                                             start=True, stop=True)
                            if oc == 0:
                                obs[(pair, t)] = ob_pool.tile(
                                    [128, C], F32, tag="ob",
                                    name=f"ob{pair}_{t}")
                            ob = obs[(pair, t)]
                            nc.vector.tensor_scalar_mul(ob[:, osl], pp0[:],
                                                        rec0[:, t:t + 1])
                            nc.vector.scalar_tensor_tensor(
                                ob[:, osl], pp1[:], rec1[:, t:t + 1],
                                ob[:, osl], op0=mult, op1=add)
                            if oc == 1:
                                nc.sync.dma_start(out_r[8 * pair + t],
                                                  obs.pop((pair, t))[:])
                        return closure

                    for n in range(NSTEP + LAG):
                        if n < NSTEP:
                            emit_st(n)
                        if n >= LAG:
                            m = n - LAG
                            emit_av(m)
                            b, j = m // NJ, m % NJ
                            if j == NJ - 1:
                                emit_block_tail(b)
                                if b % 2 == 1:
                                    pair = b // 2
                                    for t in range(8):
                                        for oc in range(2):
                                            spread.append(
                                                make_proj(pair, t, oc))
                            elif spread:
                                spread.pop(0)()
                    while spread:
                        spread.pop(0)()
    nc.finalize()
    return nc


def _colq(h):
    return slice(h * D, (h + 1) * D)


def _colk(h):
    base = h * D if h < 8 else 2 * 512 + (h - 8) * D
    return slice(base, base + D)


def _colv(h):
    base = 512 + h * D if h < 8 else 3 * 512 + (h - 8) * D
    return slice(base, base + D)


def _pmajor(a):
    """[1024, m] -> [128, 8, m] with row index = t*128 + p."""
    m = a.shape[1]
    return np.ascontiguousarray(
        a.reshape(NCT, 128, m).transpose(1, 0, 2)).astype(np.float16)


def make_in_maps(x, Wq, Wkv, Wproj):
    x = np.asarray(x, np.float32).reshape(N, C)
    Wq = np.asarray(Wq, np.float32)
    Wkv = np.asarray(Wkv, np.float32)
    Wproj = np.asarray(Wproj, np.float32)
    xt = _pmajor(np.ascontiguousarray(x.T))  # [128, 8, 4096] fp16
    in_maps = []
    for core in range(NCORES):
        h0, h1 = 2 * core, 2 * core + 1
        in_maps.append({
            "xt": xt,
            "wq": _pmajor(np.concatenate(
                [Wq[:, _colq(h0)], Wq[:, _colq(h1)]], axis=1)),
            "wk": _pmajor(np.concatenate(
                [Wkv[:, _colk(h0)], Wkv[:, _colk(h1)]], axis=1)),
            "wv": _pmajor(np.concatenate(
                [Wkv[:, _colv(h0)], Wkv[:, _colv(h1)]], axis=1)),
            "wpa": np.ascontiguousarray(
                Wproj[h0 * D:(h0 + 1) * D, :]).astype(ml_dtypes.bfloat16),
            "wpb": np.ascontiguousarray(
                Wproj[h1 * D:(h1 + 1) * D, :]).astype(ml_dtypes.bfloat16),
        })
    return in_maps


_NC = None


def _get_nc():
    global _NC
    if _NC is None:
        _NC = build_nc()
    return _NC


def run_spmd(in_maps, **kwargs):
    return run_bass_kernel_spmd(_get_nc(), in_maps, list(range(NCORES)), **kwargs)


def kernel(x, Wq, Wkv, Wproj, bproj, H=None, W=None, **_unused):
    in_maps = make_in_maps(x, Wq, Wkv, Wproj)
    res = run_spmd(in_maps)
    acc = np.zeros((N, C), np.float64)
    for r in res.results:
        acc += r["out"]
    out = acc.astype(np.float32) + np.asarray(bproj, np.float32)[None, :]
    return out.reshape(1, N, C)


if __name__ == "__main__":
    nc = build_nc()
    print("built ok")
